# revision 34
# baseline (speedup 1.0000x reference)
"""Trainium2 Bass kernel for nn_Attention_48876727828718.

RBF-kernel causal attention with per-head full-rank projections:
  xn = LayerNorm(x) * ln_w
  Q/K/V = xn @ W_{q,k,v}[h]          (per head, [S,E]@[E,E])
  scores = exp(-gamma_h * ||q_i - k_j||^2 / sqrt(E)) * causal
  out = (scores @ V  concat heads) @ W_o.T

Sharding: B(2) x H(8) = 16 (b,h) pairs over 8 cores; core c handles
batch b = c//4 and heads {2*(c%4), 2*(c%4)+1}.  Host sums the per-head
output halves and the 4 partial outputs per batch.

Factorized device algorithm per (b, h), flash-style (scores never touch
HBM).  With gs = 2*gamma/sqrt(E):
  scores_jq = exp(gs*K_j.Q_q) * bfac_j * aq_q
  bfac_j = exp(-gs*k2_j/2)   -> folded into VW on host (xnb = xn*bfac)
  aq_q   = exp(-gs*q2_q/2)   -> per-column scale on the OT accumulator
  gs     -> folded into W_q on host, so the device exp has NO scale/bias

Structure per q-super (processed 3,2,1,0):
  - T tile [128, 1024] per jb holds the HEAD PAIR: h0's K.Q chunk in psum
    bank 0 (cols 0:512), h1's in bank 1 (cols 512:1024).  The two row-
    packed T matmuls (h0 rows 0:63, h1 rows 64:127, K=64) drain into
    DIFFERENT banks so they run concurrently (same-bank concurrent PE
    drains fault).  3-deep psum pipeline.
  - exp per tile on ACT (hw Exp) or DVE (int16-Schraudolph:
    bits = rint(128*(127-c) + 128*log2e*z), bitcast bf16); short diag
    tiles use two exp instructions so unwritten psum is never read.
  - causal mask via gpsimd affine_select on each diag chunk's head.
  - OT accumulates col-packed (h0 -> psum rows 0:63, h1 -> 64:127) with
    memset + start=False (bank-wide has_written clears can't race).
  - pass end: OUT = OT * AQ (per-column aq, DVE), DMA out; host adds the
    two head halves.
"""

import math

import numpy as np

B, S, E, H = 2, 2048, 64, 8
EPS = 1e-5
NCORES = 8
NB = S // 128  # 16 j blocks
NQ = S // 512  # 4 q supers

_BUILT = {}


def _build():
    """Build + compile the single-core Bass program (same NEFF for all cores)."""
    from contextlib import ExitStack

    import concourse.bass as bass
    import concourse.mybir as mybir
    import concourse.tile as tile
    from concourse import bacc

    fp32 = mybir.dt.float32
    f32r = mybir.dt.float32r
    bf16 = mybir.dt.bfloat16
    fp16 = mybir.dt.float16
    i16 = mybir.dt.int16
    Exp = mybir.ActivationFunctionType.Exp
    Copy = mybir.ActivationFunctionType.Copy
    mult = mybir.AluOpType.mult
    add_op = mybir.AluOpType.add
    is_ge = mybir.AluOpType.is_ge

    def mm(ap):
        return ap.bitcast(f32r)

    rr = mm  # writers of f32r matmul-feeding tiles must emit rounded values

    nc = bacc.Bacc("TRN2", target_bir_lowering=False, debug=False)

    xnt_d = nc.dram_tensor("xnt", [E, S], fp16, kind="ExternalInput").ap()
    xnb_d = nc.dram_tensor("xnb", [2, E, S], bf16, kind="ExternalInput").ap()
    wqk_d = nc.dram_tensor("wqk", [E, 4 * E], fp16, kind="ExternalInput").ap()
    wvob_d = nc.dram_tensor("wvob", [2, E, E], bf16, kind="ExternalInput").ap()
    # both heads' 64-row halves; host adds them (DVE cannot cross partitions)
    out_d = nc.dram_tensor("out", [128, S], fp32, kind="ExternalOutput").ap()

    SCH_A = 128.0 / math.log(2.0)
    SCH_B = 128.0 * (127.0 - 0.022)

    with ExitStack() as ctx:
        tc = ctx.enter_context(tile.TileContext(nc))
        const = ctx.enter_context(tc.tile_pool(name="const", bufs=1))
        sb = ctx.enter_context(tc.tile_pool(name="sb", bufs=1))
        hb = ctx.enter_context(tc.tile_pool(name="hb", bufs=1))
        texp_pool = ctx.enter_context(tc.tile_pool(name="texp", bufs=8))
        ps_T = ctx.enter_context(tc.tile_pool(name="psT", bufs=3, space="PSUM"))
        ps_ot = ctx.enter_context(tc.tile_pool(name="psot", bufs=2, space="PSUM"))

        # ---- constants ----
        zero_col = const.tile([128, 1], fp32)
        nc.gpsimd.memset(zero_col, 0.0)
        nc.const_aps.aps[(fp32, 0.0)] = zero_col

        # ---- inputs: everything fp16/bf16, host pre-rounded; the bulk
        # xnb/wvo transfers trigger from the (idle) gpsimd queue so the
        # projection chain is never queued behind them.  xnt is split into
        # per-super slices so the first projection can start early. ----
        wqk_sb = const.tile([E, 4 * E], fp16)
        nc.sync.dma_start(wqk_sb, wqk_d)
        wq_sb = wqk_sb[:, 0 : 2 * E]
        wk_sb = wqk_sb[:, 2 * E : 4 * E]
        wvo_sb = const.tile([E, 2 * E], bf16)
        nc.gpsimd.dma_start(
            wvo_sb.rearrange("e (h f) -> e h f", h=2), wvob_d.transpose([1, 0, 2])
        )
        xnT = sb.tile([E, S], fp16)
        for c4 in (0, 3, 1, 2):
            csl = slice(c4 * 512, (c4 + 1) * 512)
            nc.sync.dma_start(xnT[:, csl], xnt_d[:, csl])
        xnb = {}
        for h in range(2):
            xnb[h] = sb.tile([E, S], bf16, name=f"xnb{h}")
            for half in range(2):
                hs = slice(half * 1024, (half + 1) * 1024)
                nc.gpsimd.dma_start(xnb[h][:, hs], xnb_d[h][:, hs])
        # constant lower-tri mask (1 where col >= row) for DVE-side masking
        tri = const.tile([128, 128], bf16)
        nc.gpsimd.memset(tri, 1.0)
        nc.gpsimd.affine_select(
            out=tri,
            in_=tri,
            pattern=[[1, 128]],
            compare_op=is_ge,
            fill=0.0,
            base=0,
            channel_multiplier=-1,
        )

        OUTsb = sb.tile([128, S], fp32)

        # ---- projections: QT/KT stacked [128, S] bf16 (h0 rows 0:63, h1
        # rows 64:127).  One matmul projects BOTH heads (lhsT [64, 128] =
        # both heads' weights).  Ordered so the main loop (super 3, jb
        # ascending) can start after the first two copies land.
        QTs = hb.tile([128, S], bf16, name="QTs", tag="qts")
        KTs = hb.tile([128, S], bf16, name="KTs", tag="kts")
        proj_work = [
            (KTs, wk_sb, 0),
            (QTs, wq_sb, 3),
            (KTs, wk_sb, 1),
            (KTs, wk_sb, 2),
            (KTs, wk_sb, 3),
            (QTs, wq_sb, 2),
            (QTs, wq_sb, 1),
            (QTs, wq_sb, 0),
        ]
        for idx, (dst, w_sb, c4) in enumerate(proj_work):
            pp = ps_ot.tile([128, 512], fp32, name=f"pp{idx}", tag="ot")
            nc.tensor.matmul(
                pp,
                w_sb,
                xnT[:, c4 * 512 : (c4 + 1) * 512],
                start=True,
                stop=True,
            )
            dslice = dst[:, c4 * 512 : (c4 + 1) * 512]
            if idx % 2 == 0:
                nc.scalar.activation(dslice, pp, Copy)
            else:
                nc.vector.tensor_copy(dslice, pp)

        # ---- VW' = (xn * bfac) @ (W_v @ W_o_blk^T) per head, bf16.
        # Group g (jb 4g..4g+3) for both heads; g0 is emitted up front,
        # g1..g3 are interleaved into the first main-loop pairs. ----
        VWs = {}
        for h in range(2):
            VWs[h] = hb.tile([128, NB * E], bf16, name=f"VW{h}", tag=f"vw{h}")

        def emit_vw_group(g, pool=None):
            for h in range(2):
                pv = (pool or ps_ot).tile([128, 256], fp32, name=f"pv{h}{g}", tag="ot" if pool is None else "T")
                for k in range(4):
                    jb = 4 * g + k
                    nc.tensor.matmul(
                        pv[:, k * E : (k + 1) * E],
                        xnb[h][:, jb * 128 : (jb + 1) * 128],
                        wvo_sb[:, h * E : (h + 1) * E],
                        start=True,
                        stop=True,
                    )
                if (h + g) % 2 == 0:
                    nc.vector.tensor_copy(VWs[h][:, g * 256 : (g + 1) * 256], pv)
                else:
                    nc.scalar.activation(
                        VWs[h][:, g * 256 : (g + 1) * 256], pv, Copy
                    )

        # ---- main loop: super groups [(3,),(2,),(1,0)]; the last group
        # interleaves its two supers' tiles so the short-tail pipeline
        # stays deep.  Tiles alternate ACT/DVE exp strictly. ----
        tile_idx = 0
        for group in ((3,), (2,), (1, 0)):
            OTps = {}
            for qs in group:
                OTps[qs] = ps_ot.tile([128, 512], fp32, name=f"ot{qs}", tag="ot")
                # zero the accumulator; all OT matmuls use start=False so
                # the col-packed head regions never race on a bank-wide
                # has_written clear (accumulate-onto-zero == overwrite).
                nc.vector.memset(OTps[qs], 0.0)

            # per-super jb lists, interleaved across the group
            per_qs = {
                qs: [(qs, jb, 0, 512) for jb in range(4 * qs)]
                + [(qs, 4 * qs + r, 128 * r, 512 - 128 * r) for r in range(4)]
                for qs in group
            }
            items = []
            k = 0
            while any(per_qs.values()):
                for qs in group:
                    if per_qs[qs]:
                        it = per_qs[qs].pop(0)
                        items.append(it + (len(per_qs[qs]) == 0,))

            def emit_ot(item):
                texp_, qs_, jb_, dead_, w_, last_ = item
                for h in range(2):
                    nc.tensor.matmul(
                        OTps[qs_][h * E : (h + 1) * E, dead_:512],
                        VWs[h][:, jb_ * E : (jb_ + 1) * E],
                        texp_[:, h * 512 : h * 512 + w_],
                        start=False,
                        stop=(last_ and h == 1),
                    )

            pend = []
            for qs, jb, dead, w, is_last in items:
                q0 = 512 * qs
                tch = ps_T.tile([128, 1024], fp32, name=f"tt{tile_idx}", tag="T")
                for h in range(2):
                    p0 = h * 64
                    nc.tensor.matmul(
                        tch[:, h * 512 : h * 512 + w],
                        KTs[p0 : p0 + 64, jb * 128 : (jb + 1) * 128],
                        QTs[p0 : p0 + 64, q0 + dead : q0 + 512],
                        start=True,
                        stop=True,
                    )
                # interleave the VW group builds into the first tiles
                if tile_idx in (0, 2, 4, 6):
                    emit_vw_group(tile_idx // 2, pool=ps_T)
                texp = texp_pool.tile([128, 1024], bf16, name=f"tx{tile_idx}", tag="te")
                use_dve = tile_idx % 2 == 1  # strict ACT/DVE alternation
                spans = [(0, w), (512, w)] if w < 512 else [(0, 1024)]
                for sp_off, sp_w in spans:
                    if use_dve:
                        nc.vector.tensor_scalar(
                            texp.bitcast(i16)[:, sp_off : sp_off + sp_w],
                            tch[:, sp_off : sp_off + sp_w],
                            SCH_A,
                            SCH_B,
                            mult,
                            op1=add_op,
                        )
                    else:
                        nc.scalar.activation(
                            texp[:, sp_off : sp_off + sp_w],
                            tch[:, sp_off : sp_off + sp_w],
                            Exp,
                        )
                if jb // 4 == qs:
                    # causal mask on each chunk's first 128 cols: local col
                    # t is q = 128*jb + t vs j = 128*jb + p.  h0 on the
                    # gpsimd queue, h1 on DVE (tri-mask multiply) so one
                    # queue never serializes both.
                    for hh in range(2):
                        nc.gpsimd.affine_select(
                            out=texp[:, hh * 512 : hh * 512 + 128],
                            in_=texp[:, hh * 512 : hh * 512 + 128],
                            pattern=[[1, 128]],
                            compare_op=is_ge,
                            fill=0.0,
                            base=0,
                            channel_multiplier=-1,
                        )
                pend.append((texp, qs, jb, dead, w, is_last))
                if len(pend) > 2:
                    emit_ot(pend.pop(0))
                tile_idx += 1
            for item in pend:
                emit_ot(item)
            pend = []
            for gi, qs in enumerate(group):
                qsl = slice(512 * qs, 512 * qs + 512)
                # evacuate psum (aq scaling is applied on the host)
                if gi % 2 == 0:
                    nc.vector.tensor_copy(OUTsb[:, qsl], OTps[qs])
                else:
                    nc.scalar.activation(OUTsb[:, qsl], OTps[qs], Copy)
                nc.sync.dma_start(out_d[:, qsl], OUTsb[:, qsl])

    nc.compile()
    return nc


def _get_nc():
    if 0 not in _BUILT:
        _BUILT[0] = _build()
    return _BUILT[0]


def _prep_inputs(x, ln_w, W_q, W_k, W_v, W_o, gamma):
    """Host-side input prep: fold weights, stat scales, shard per core."""
    import ml_dtypes

    x = np.asarray(x, np.float32)
    ln_w = np.asarray(ln_w, np.float32)
    W_q = np.asarray(W_q, np.float32)
    W_k = np.asarray(W_k, np.float32)
    W_v = np.asarray(W_v, np.float32)
    W_o = np.asarray(W_o, np.float32)
    gamma = np.asarray(gamma, np.float32).reshape(H)

    # fold ln_w into projection weights; fold W_o into W_v
    lw = ln_w[None, :, None]  # [1, E, 1] scale on contraction dim e
    Wq = (W_q * lw).astype(np.float32)
    Wk = (W_k * lw).astype(np.float32)
    Wv = (W_v * lw).astype(np.float32)
    Wo_blk = W_o.reshape(E, H, E).transpose(1, 0, 2)  # [H, e_out, f]
    Wvo = np.einsum("hef,hof->heo", Wv.astype(np.float64), Wo_blk.astype(np.float64))
    Wvo = Wvo.astype(np.float32)  # [H, e, e_out]
    gs = (2.0 * gamma / np.sqrt(E)).astype(np.float32)  # exp scale per head

    # host-computed per-(b,h) stats
    mu = x.mean(-1, keepdims=True)
    var = ((x - mu) ** 2).mean(-1, keepdims=True)
    xn = (x - mu) / np.sqrt(var + EPS)  # ln_w folded into weights
    Qh = np.einsum("bse,hef->bhsf", xn, Wq)  # [B,H,S,E]
    Kh = np.einsum("bse,hef->bhsf", xn, Wk)
    q2 = (Qh * Qh).sum(-1)  # [B,H,S]
    k2 = (Kh * Kh).sum(-1)

    in_maps = []
    for c in range(NCORES):
        b = c // 4
        h0 = 2 * (c % 4)
        xnt = np.ascontiguousarray(xn[b].T.astype(np.float32))  # [E, S]
        xnb = np.zeros((2, E, S), np.float32)
        wqk = np.zeros((E, 4 * E), np.float32)  # [e, (h f)] wq | wk packed
        for h in range(2):
            g = gs[h0 + h]
            bfac = np.exp(-0.5 * g * k2[b, h0 + h]).astype(np.float32)  # [S]
            xnb[h] = xnt * bfac[None, :]
            wqk[:, h * E : (h + 1) * E] = g * Wq[h0 + h]  # gs folded in
            wqk[:, (2 + h) * E : (3 + h) * E] = Wk[h0 + h]
        in_maps.append(
            {
                "xnt": xnt.astype(np.float16),
                "xnb": xnb.astype(ml_dtypes.bfloat16),
                "wqk": wqk.astype(np.float16),
                "wvob": np.ascontiguousarray(
                    Wvo[h0 : h0 + 2].astype(ml_dtypes.bfloat16)
                ),
            }
        )
    return _aq(q2, gs), in_maps


def _aq(q2, gs):
    """aq[b, h, q] = exp(-gs_h*q2/2), applied host-side to the output."""
    return np.exp(-0.5 * gs[None, :, None] * q2).astype(np.float32)


def kernel(x, ln_w, W_q, W_k, W_v, W_o, gamma):
    from concourse import bass_utils

    nc = _get_nc()
    aq, in_maps = _prep_inputs(x, ln_w, W_q, W_k, W_v, W_o, gamma)
    res = bass_utils.run_bass_kernel_spmd(nc, in_maps, core_ids=list(range(NCORES)))

    out = np.zeros((B, S, E), np.float32)
    for c in range(NCORES):
        b, h0 = c // 4, 2 * (c % 4)
        o = res.results[c]["out"]  # [128, S]: rows 0:64 = h0, 64:128 = h1
        out[b] += (o[0:64] * aq[b, h0][None, :]).T
        out[b] += (o[64:128] * aq[b, h0 + 1][None, :]).T
    return out
